# revision 7
# baseline (speedup 1.0000x reference)
# Trainium2 Bass kernel for nn_CausalSelfAttention_58239756533763.
#
# Sharding: tensor-parallel over heads. 16 heads / 8 cores = 2 heads per core.
# Each core computes q/k/v projections for its 2 heads (all 4 batches),
# attention with XL memory, and a partial output projection against its
# 128 rows of Wo. The host sums the 8 partial outputs (the "all-reduce") and
# concatenates the per-core k/v slices for the kv_to_add_xl output.
#
# On-chip layout: everything flows through the transposed ("feature on
# partitions") layout so no on-device transposes of activations are needed:
#   x^T, q^T, k^T are [feature, token]; v is [token, feature].
#   Scores are computed as S^T = [j, i] tiles; softmax sums over j come from
#   a ones-column appended to v in the P@V matmul (row 64 of the PSUM result).
#   The causal mask and the relative-position bias are folded together on the
#   host into a premasked rel^T input (-1e30 at masked (j, i) positions).
# Matmuls run as float32r (one-pass FP22 multiply, fp32 accumulate).

import numpy as np

B, T, MXL, E, H, D = 4, 1024, 1024, 1024, 16, 64
SCALE = D ** (-0.5)
NCORES = 8
HPC = H // NCORES          # heads per core = 2
HDC = HPC * D              # head-dim columns per core = 128
BT = B * T                 # 4096 tokens
J = MXL + T                # 2048 key positions
NEG = -1.0e30
P = 128

_CACHE = {}


def _build_program():
    import concourse.mybir as mybir
    import concourse.tile as tile
    from concourse import bacc
    from concourse.masks import make_identity

    fp32 = mybir.dt.float32
    f32r = mybir.dt.float32r
    AF = mybir.ActivationFunctionType

    nc = bacc.Bacc("TRN2", target_bir_lowering=False, debug=False,
                   num_devices=NCORES)

    xT = nc.dram_tensor("xT", [E, BT], f32r, kind="ExternalInput")
    wq = nc.dram_tensor("wq", [E, HDC], f32r, kind="ExternalInput")
    wk = nc.dram_tensor("wk", [E, HDC], f32r, kind="ExternalInput")
    wv = nc.dram_tensor("wv", [E, HDC], f32r, kind="ExternalInput")
    wo = nc.dram_tensor("wo", [HDC, E], f32r, kind="ExternalInput")
    bqd = nc.dram_tensor("bqd", [HDC, 1], fp32, kind="ExternalInput")
    bkd = nc.dram_tensor("bkd", [HDC, 1], fp32, kind="ExternalInput")
    bvd = nc.dram_tensor("bvd", [1, HDC], fp32, kind="ExternalInput")
    kxlT = nc.dram_tensor("kxlT", [HDC, BT], f32r, kind="ExternalInput")
    vxl = nc.dram_tensor("vxl", [BT, HDC], f32r, kind="ExternalInput")
    relT = nc.dram_tensor("relT", [HPC, J, T], fp32, kind="ExternalInput")
    outp = nc.dram_tensor("outp", [BT, E], fp32, kind="ExternalOutput")
    kvp = nc.dram_tensor("kvp", [BT, 2, HDC], fp32, kind="ExternalOutput")

    NJT = J // P               # 16 j-tiles per (b, h)
    IR = 512                   # i-range (query block, matmul free dim)
    PVW = D + 1                # 65: v columns + ones column per head

    with tile.TileContext(nc) as tc:
        with tc.tile_pool(name="const", bufs=1) as constp, \
             tc.tile_pool(name="big", bufs=1) as bigp:
            ident = constp.tile([P, P], fp32)
            make_identity(nc, ident[:])
            bq_sb = constp.tile([HDC, 1], fp32)
            bk_sb = constp.tile([HDC, 1], fp32)
            bv_row = constp.tile([1, HDC], fp32)
            bv_bc = constp.tile([P, HDC], fp32)
            wo_sb = constp.tile([HDC, E], f32r)
            nc.sync.dma_start(bq_sb[:], bqd.ap())
            nc.sync.dma_start(bk_sb[:], bkd.ap())
            nc.sync.dma_start(bv_row[:], bvd.ap())
            nc.gpsimd.partition_broadcast(bv_bc[:], bv_row[:])
            nc.sync.dma_start(wo_sb[:], wo.ap())

            kT = bigp.tile([P, B, J], f32r)        # k^T incl. XL memory
            vpv = bigp.tile([P, B, NJT, 2 * PVW], f32r)  # [vA|1|vB|1] per j-tile
            qT = bigp.tile([P, B, T], f32r)
            qkvn = bigp.tile([P, B, T], f32r)      # normalized attn out (^T)
            rel_sb = bigp.tile([P, NJT, 512], fp32)  # rel^T, one (h, i-range)

            # ones columns used to produce softmax denominators in P@V
            nc.vector.memset(vpv[:, :, :, D:D + 1].bitcast(fp32), 1.0)
            nc.vector.memset(vpv[:, :, :, PVW + D:PVW + D + 1].bitcast(fp32),
                             1.0)

            # XL memory: keys (transposed on host) and values
            nc.sync.dma_start(
                kT[:, :, 0:MXL],
                kxlT.ap().rearrange("p (b t) -> p b t", b=B))
            vxl_r = vxl.ap().rearrange("(b j p) c -> p b j c", b=B, p=P)
            for b in range(B):
                nc.sync.dma_start(vpv[:, b, 0:MXL // P, 0:D],
                                  vxl_r[:, b, :, 0:D])
                nc.sync.dma_start(vpv[:, b, 0:MXL // P, PVW:PVW + D],
                                  vxl_r[:, b, :, D:2 * D])

            # ---- Phase 1: projections --------------------------------------
            with tc.tile_pool(name="w3", bufs=1) as w3p, \
                 tc.tile_pool(name="xt", bufs=2) as xtp, \
                 tc.tile_pool(name="ps1", bufs=2, space="PSUM") as ps1, \
                 tc.tile_pool(name="sb1", bufs=3) as sb1:
                wq_sb = w3p.tile([P, E // P, HDC], f32r)
                wk_sb = w3p.tile([P, E // P, HDC], f32r)
                wv_sb = w3p.tile([P, E // P, HDC], f32r)
                nc.sync.dma_start(
                    wq_sb[:], wq.ap().rearrange("(a p) m -> p a m", p=P))
                nc.sync.dma_start(
                    wk_sb[:], wk.ap().rearrange("(a p) m -> p a m", p=P))
                nc.sync.dma_start(
                    wv_sb[:], wv.ap().rearrange("(a p) m -> p a m", p=P))

                for b in range(B):
                    for tr in range(T // IR):
                        col0 = b * T + tr * IR
                        xt = xtp.tile([P, E // P, IR], f32r)
                        nc.sync.dma_start(
                            xt[:],
                            xT.ap()[:, col0:col0 + IR]
                              .rearrange("(a p) t -> p a t", p=P))
                        qps = ps1.tile([P, IR], fp32, tag="qps")
                        for e in range(E // P):
                            nc.tensor.matmul(qps[:], wq_sb[:, e, :],
                                             xt[:, e, :],
                                             start=(e == 0),
                                             stop=(e == E // P - 1))
                        nc.scalar.activation(
                            qT[:, b, tr * IR:(tr + 1) * IR], qps[:],
                            AF.Identity, bias=bq_sb[:])
                        kps = ps1.tile([P, IR], fp32, tag="kps")
                        for e in range(E // P):
                            nc.tensor.matmul(kps[:], wk_sb[:, e, :],
                                             xt[:, e, :],
                                             start=(e == 0),
                                             stop=(e == E // P - 1))
                        nc.scalar.activation(
                            kT[:, b, MXL + tr * IR:MXL + (tr + 1) * IR],
                            kps[:], AF.Identity, bias=bk_sb[:])

                        for s in range(IR // P):
                            t0 = tr * IR + s * P
                            vps = ps1.tile([P, P], fp32, tag="vps")
                            for e in range(E // P):
                                nc.tensor.matmul(
                                    vps[:], xt[:, e, s * P:(s + 1) * P],
                                    wv_sb[:, e, :],
                                    start=(e == 0), stop=(e == E // P - 1))
                            jt = MXL // P + tr * (IR // P) + s
                            vsb = sb1.tile([P, P], fp32, tag="vsb")
                            nc.vector.tensor_add(vsb[:], vps[:], bv_bc[:])
                            nc.vector.tensor_copy(vpv[:, b, jt, 0:D],
                                                  vsb[:, 0:D])
                            nc.vector.tensor_copy(vpv[:, b, jt, PVW:PVW + D],
                                                  vsb[:, D:2 * D])
                            nc.sync.dma_start(
                                kvp.ap()[b * T + t0:b * T + t0 + P, 1, :],
                                vsb[:])
                            # k natural (current chunk) for the kv output
                            ktp = ps1.tile([P, P], fp32, tag="ktp")
                            nc.tensor.transpose(
                                ktp[:],
                                kT[:, b, MXL + t0:MXL + t0 + P].bitcast(fp32),
                                ident[:])
                            ksb = sb1.tile([P, P], fp32, tag="ksb")
                            nc.vector.tensor_copy(ksb[:], ktp[:])
                            nc.sync.dma_start(
                                kvp.ap()[b * T + t0:b * T + t0 + P, 0, :],
                                ksb[:])

            # ---- Phase 2: attention ----------------------------------------
            with tc.tile_pool(name="lg", bufs=3) as lgp, \
                 tc.tile_pool(name="ex", bufs=3) as exp_, \
                 tc.tile_pool(name="sm", bufs=2) as smp, \
                 tc.tile_pool(name="psS", bufs=2, space="PSUM") as psS, \
                 tc.tile_pool(name="psPV", bufs=2, space="PSUM") as psPV:
                for h in range(HPC):
                    h0 = h * D
                    for ir in range(T // IR):
                        i0 = ir * IR
                        nj = (MXL + i0 + IR) // P  # visible j-tiles
                        nc.sync.dma_start(
                            rel_sb[:],
                            relT.ap()[h][:, i0:i0 + IR]
                                .rearrange("(a p) i -> p a i", p=P))
                        for b in range(B):
                            pvps = psPV.tile([PVW, IR], fp32, tag="pv")
                            for jt in range(nj):
                                sps = psS.tile([P, IR], fp32, tag="s")
                                nc.tensor.matmul(
                                    sps[:],
                                    kT[h0:h0 + D, b, jt * P:(jt + 1) * P],
                                    qT[h0:h0 + D, b, i0:i0 + IR],
                                    start=True, stop=True)
                                lg = lgp.tile([P, IR], fp32, tag="lg")
                                nc.vector.tensor_add(
                                    lg[:], sps[:], rel_sb[:, jt, :])
                                ex = exp_.tile([P, IR], f32r, tag="ex")
                                nc.scalar.activation(ex[:], lg[:], AF.Exp,
                                                     scale=SCALE)
                                nc.tensor.matmul(
                                    pvps[:],
                                    vpv[:, b, jt, h * PVW:(h + 1) * PVW],
                                    ex[:],
                                    start=(jt == 0), stop=(jt == nj - 1))
                            rr = smp.tile([1, IR], fp32, tag="rr")
                            nc.vector.reciprocal(rr[:], pvps[D:D + 1, :])
                            rb = smp.tile([D, IR], fp32, tag="rb")
                            nc.gpsimd.partition_broadcast(rb[:], rr[:])
                            nc.vector.tensor_mul(
                                qkvn[h0:h0 + D, b, i0:i0 + IR],
                                pvps[0:D, :], rb[:])

            # ---- Phase 3: output projection (partial) ----------------------
            with tc.tile_pool(name="psO", bufs=2, space="PSUM") as psO, \
                 tc.tile_pool(name="osb", bufs=3) as osbp:
                for b in range(B):
                    for tt in range(T // P):
                        t0 = tt * P
                        for eh in range(E // 512):
                            ops = psO.tile([P, 512], fp32, tag="o")
                            nc.tensor.matmul(
                                ops[:], qkvn[:, b, t0:t0 + P],
                                wo_sb[:, eh * 512:(eh + 1) * 512],
                                start=True, stop=True)
                            osb = osbp.tile([P, 512], fp32, tag="osb")
                            nc.vector.tensor_copy(osb[:], ops[:])
                            nc.sync.dma_start(
                                outp.ap()[b * T + t0:b * T + t0 + P,
                                          eh * 512:(eh + 1) * 512],
                                osb[:])

    nc.compile()
    return nc


def _get_program():
    if "nc" not in _CACHE:
        _CACHE["nc"] = _build_program()
    return _CACHE["nc"]


def _prep_inputs(x, xl_memory, relative_positions, Wq, bq, Wk, bk, Wv, bv):
    """Host-side sharding/layout prep. Returns per-core input maps."""
    x = np.ascontiguousarray(np.asarray(x, np.float32))
    xl = np.asarray(xl_memory, np.float32)
    rel = np.asarray(relative_positions, np.float32)

    xT = np.ascontiguousarray(
        x.reshape(BT, E).T)                      # [E, BT]

    # causal mask folded into rel^T: mask (j >= i + MXL + 1) -> -1e30
    jj = np.arange(J, dtype=np.int64)[:, None]
    ii = np.arange(T, dtype=np.int64)[None, :]
    maskT = jj >= (ii + MXL + 1)                 # [J, T]

    in_maps = []
    for c in range(NCORES):
        cs = slice(c * HDC, (c + 1) * HDC)
        relc = np.ascontiguousarray(
            rel[c * HPC:(c + 1) * HPC].transpose(0, 2, 1))  # [HPC, J, T]
        relc[:, maskT] = NEG
        in_maps.append({
            "xT": xT,
            "wq": np.ascontiguousarray(Wq[:, cs] * SCALE),
            "wk": np.ascontiguousarray(Wk[:, cs]),
            "wv": np.ascontiguousarray(Wv[:, cs]),
            "bqd": np.ascontiguousarray(
                (bq[cs] * SCALE).reshape(HDC, 1).astype(np.float32)),
            "bkd": np.ascontiguousarray(
                np.asarray(bk[cs], np.float32).reshape(HDC, 1)),
            "bvd": np.ascontiguousarray(
                np.asarray(bv[cs], np.float32).reshape(1, HDC)),
            "kxlT": np.ascontiguousarray(
                xl[:, :, 0, cs].reshape(BT, HDC).T),
            "vxl": np.ascontiguousarray(xl[:, :, 1, cs].reshape(BT, HDC)),
            "relT": relc,
        })
    return in_maps


def _run(inputs, trace=False, tmpdir=None, trace_cores=None):
    from concourse.bass_utils import run_bass_kernel_spmd

    x = np.asarray(inputs["x"], np.float32)
    xl = np.asarray(inputs["xl_memory"], np.float32)
    rel = np.asarray(inputs["relative_positions"], np.float32)
    Wq = np.asarray(inputs["Wq"], np.float32)
    bq = np.asarray(inputs["bq"], np.float32)
    Wk = np.asarray(inputs["Wk"], np.float32)
    bk = np.asarray(inputs["bk"], np.float32)
    Wv = np.asarray(inputs["Wv"], np.float32)
    bv = np.asarray(inputs["bv"], np.float32)
    Wo = np.asarray(inputs["Wo"], np.float32)
    bo = np.asarray(inputs["bo"], np.float32)

    in_maps = _prep_inputs(x, xl, rel, Wq, bq, Wk, bk, Wv, bv)
    for c in range(NCORES):
        cs = slice(c * HDC, (c + 1) * HDC)
        in_maps[c]["wo"] = np.ascontiguousarray(Wo[cs, :])

    nc = _get_program()
    kw = {}
    if trace:
        kw.update(trace=True, tmpdir=tmpdir, trace_cores=trace_cores)
    res = run_bass_kernel_spmd(nc, in_maps, list(range(NCORES)), **kw)

    out = np.zeros((BT, E), np.float32)
    kv = np.empty((B, T, 2, H * D), np.float32)
    for c in range(NCORES):
        cs = slice(c * HDC, (c + 1) * HDC)
        out += res.results[c]["outp"]
        kv[:, :, :, cs] = res.results[c]["kvp"].reshape(B, T, 2, HDC)
    out = out.reshape(B, T, E) + bo.astype(np.float32)
    return (out, kv), res


def kernel(**inputs):
    outs, _ = _run(inputs, trace=False)
    return outs


# revision 8
# speedup vs baseline: 1.2157x; 1.2157x over previous
# Trainium2 Bass kernel for nn_CausalSelfAttention_58239756533763.
#
# Sharding: tensor-parallel over heads. 16 heads / 8 cores = 2 heads per core.
# Each core computes q/k/v projections for its 2 heads (all 4 batches),
# attention with XL memory, and a partial output projection against its
# 128 rows of Wo. The host sums the 8 partial outputs (the "all-reduce" of
# the sharding hint, done at unshard time) and concatenates the per-core
# k/v slices for the kv_to_add_xl output.
#
# On-chip layout: activations flow in the transposed ("feature on
# partitions") layout so no on-device transposes are needed:
#   x^T, q^T, k^T are [feature, token]; v is [token, feature].
#   Scores are computed as S^T = [j, i] tiles. Softmax denominators come
#   from a 64-wide ones block appended to v in the P@V matmul, so the sums
#   land on 64 partitions (row-parallel reciprocal).
#   exp((S + rel) * s) = exp(S * s) * exp(rel * s): the host precomputes
#   exp(rel^T * s) in bf16 with masked (j, i) entries set to 0, so on
#   device the bias-add becomes a bf16 multiply and masking is exact.
# Precision: k/v/q projections and the output projection data path that
# feeds the kv output run as float32r (one-pass FP22); the attention
# matmuls run in bf16.

import numpy as np

B, T, MXL, E, H, D = 4, 1024, 1024, 1024, 16, 64
SCALE = D ** (-0.5)
NCORES = 8
HPC = H // NCORES          # heads per core = 2
HDC = HPC * D              # head-dim columns per core = 128
BT = B * T                 # 4096 tokens
J = MXL + T                # 2048 key positions
P = 128
IR = 512                   # i-range (query block, matmul free dim)
NJT = J // P               # 16 j-tiles
PVW = 2 * D                # per-head lhsT width in P@V: [v (64) | ones (64)]

_CACHE = {}


def _build_program():
    import concourse.mybir as mybir
    import concourse.tile as tile
    from concourse import bacc
    from concourse.masks import make_identity

    fp32 = mybir.dt.float32
    f32r = mybir.dt.float32r
    bf16 = mybir.dt.bfloat16
    AF = mybir.ActivationFunctionType

    nc = bacc.Bacc("TRN2", target_bir_lowering=False, debug=False,
                   num_devices=NCORES)

    xT = nc.dram_tensor("xT", [E, BT], f32r, kind="ExternalInput")
    wq = nc.dram_tensor("wq", [E, HDC], f32r, kind="ExternalInput")
    wk = nc.dram_tensor("wk", [E, HDC], f32r, kind="ExternalInput")
    wv = nc.dram_tensor("wv", [E, HDC], f32r, kind="ExternalInput")
    wo = nc.dram_tensor("wo", [HDC, E], bf16, kind="ExternalInput")
    bqd = nc.dram_tensor("bqd", [HDC, 1], fp32, kind="ExternalInput")
    bkd = nc.dram_tensor("bkd", [HDC, 1], fp32, kind="ExternalInput")
    bvd = nc.dram_tensor("bvd", [1, HDC], fp32, kind="ExternalInput")
    kxlT = nc.dram_tensor("kxlT", [HDC, BT], f32r, kind="ExternalInput")
    vxl = nc.dram_tensor("vxl", [BT, HDC], bf16, kind="ExternalInput")
    erel = nc.dram_tensor("erel", [HPC, J, T], bf16, kind="ExternalInput")
    outp = nc.dram_tensor("outp", [BT, E], fp32, kind="ExternalOutput")
    kvp = nc.dram_tensor("kvp", [BT, 2, HDC], fp32, kind="ExternalOutput")

    with tile.TileContext(nc) as tc:
        with tc.tile_pool(name="const", bufs=1) as constp, \
             tc.tile_pool(name="big", bufs=1) as bigp:
            ident = constp.tile([P, P], fp32)
            make_identity(nc, ident[:])
            bq_sb = constp.tile([HDC, 1], fp32)
            bk_sb = constp.tile([HDC, 1], fp32)
            bv_row = constp.tile([1, HDC], fp32)
            bv_bc = constp.tile([P, HDC], fp32)
            wo_sb = constp.tile([HDC, E], bf16)
            nc.sync.dma_start(bq_sb[:], bqd.ap())
            nc.sync.dma_start(bk_sb[:], bkd.ap())
            nc.sync.dma_start(bv_row[:], bvd.ap())
            nc.gpsimd.partition_broadcast(bv_bc[:], bv_row[:])
            nc.sync.dma_start(wo_sb[:], wo.ap())

            kT = bigp.tile([P, B, J], f32r)        # k^T incl. XL memory
            kTb = bigp.tile([P, B, J], bf16)       # bf16 copy for S^T matmul
            vpv = bigp.tile([P, B, NJT, HPC * PVW], bf16)
            qT = bigp.tile([P, B, T], bf16)
            qkvn = bigp.tile([P, B, T], bf16)      # normalized attn out (^T)

            # ones blocks produce softmax denominators on partitions 64:128
            nc.vector.memset(vpv[:, :, :, D:PVW], 1.0)
            nc.vector.memset(vpv[:, :, :, PVW + D:2 * PVW], 1.0)

            # XL memory: keys (transposed on host) and values
            nc.sync.dma_start(
                kT[:, :, 0:MXL],
                kxlT.ap().rearrange("p (b t) -> p b t", b=B))
            vxl_r = vxl.ap().rearrange("(b j p) c -> p b j c", b=B, p=P)
            for b in range(B):
                nc.sync.dma_start(vpv[:, b, 0:MXL // P, 0:D],
                                  vxl_r[:, b, :, 0:D])
                nc.sync.dma_start(vpv[:, b, 0:MXL // P, PVW:PVW + D],
                                  vxl_r[:, b, :, D:2 * D])

            # ---- Phase 1: projections --------------------------------------
            with tc.tile_pool(name="w3", bufs=1) as w3p, \
                 tc.tile_pool(name="xt", bufs=2) as xtp, \
                 tc.tile_pool(name="ps1", bufs=2, space="PSUM") as ps1, \
                 tc.tile_pool(name="sb1", bufs=3) as sb1:
                wq_sb = w3p.tile([P, E // P, HDC], f32r)
                wk_sb = w3p.tile([P, E // P, HDC], f32r)
                wv_sb = w3p.tile([P, E // P, HDC], f32r)
                nc.sync.dma_start(
                    wq_sb[:], wq.ap().rearrange("(a p) m -> p a m", p=P))
                nc.sync.dma_start(
                    wk_sb[:], wk.ap().rearrange("(a p) m -> p a m", p=P))
                nc.sync.dma_start(
                    wv_sb[:], wv.ap().rearrange("(a p) m -> p a m", p=P))

                for b in range(B):
                    for tr in range(T // IR):
                        col0 = b * T + tr * IR
                        xt = xtp.tile([P, E // P, IR], f32r)
                        nc.sync.dma_start(
                            xt[:],
                            xT.ap()[:, col0:col0 + IR]
                              .rearrange("(a p) t -> p a t", p=P))
                        qps = ps1.tile([P, IR], fp32, tag="qps")
                        for e in range(E // P):
                            nc.tensor.matmul(qps[:], wq_sb[:, e, :],
                                             xt[:, e, :],
                                             start=(e == 0),
                                             stop=(e == E // P - 1))
                        nc.scalar.activation(
                            qT[:, b, tr * IR:(tr + 1) * IR], qps[:],
                            AF.Identity, bias=bq_sb[:])
                        kps = ps1.tile([P, IR], fp32, tag="kps")
                        for e in range(E // P):
                            nc.tensor.matmul(kps[:], wk_sb[:, e, :],
                                             xt[:, e, :],
                                             start=(e == 0),
                                             stop=(e == E // P - 1))
                        nc.scalar.activation(
                            kT[:, b, MXL + tr * IR:MXL + (tr + 1) * IR],
                            kps[:], AF.Identity, bias=bk_sb[:])

                        for s in range(IR // P):
                            t0 = tr * IR + s * P
                            vps = ps1.tile([P, P], fp32, tag="vps")
                            for e in range(E // P):
                                nc.tensor.matmul(
                                    vps[:], xt[:, e, s * P:(s + 1) * P],
                                    wv_sb[:, e, :],
                                    start=(e == 0), stop=(e == E // P - 1))
                            jt = MXL // P + tr * (IR // P) + s
                            vsb = sb1.tile([P, P], fp32, tag="vsb")
                            nc.vector.tensor_add(vsb[:], vps[:], bv_bc[:])
                            nc.vector.tensor_copy(vpv[:, b, jt, 0:D],
                                                  vsb[:, 0:D])
                            nc.vector.tensor_copy(vpv[:, b, jt, PVW:PVW + D],
                                                  vsb[:, D:2 * D])
                            nc.sync.dma_start(
                                kvp.ap()[b * T + t0:b * T + t0 + P, 1, :],
                                vsb[:])
                            # k natural (current chunk) for the kv output
                            ktp = ps1.tile([P, P], fp32, tag="ktp")
                            nc.tensor.transpose(
                                ktp[:],
                                kT[:, b, MXL + t0:MXL + t0 + P].bitcast(fp32),
                                ident[:])
                            ksb = sb1.tile([P, P], fp32, tag="ksb")
                            nc.vector.tensor_copy(ksb[:], ktp[:])
                            nc.sync.dma_start(
                                kvp.ap()[b * T + t0:b * T + t0 + P, 0, :],
                                ksb[:])
                    # bf16 copy of this batch's keys for the S^T matmul
                    nc.vector.tensor_copy(kTb[:, b, :], kT[:, b, :])

            # ---- Phase 2: attention ----------------------------------------
            with tc.tile_pool(name="er", bufs=2) as erp, \
                 tc.tile_pool(name="exs", bufs=3) as exsp, \
                 tc.tile_pool(name="pex", bufs=3) as pexp, \
                 tc.tile_pool(name="sm", bufs=2) as smp, \
                 tc.tile_pool(name="psS", bufs=3, space="PSUM") as psS, \
                 tc.tile_pool(name="psPV", bufs=2, space="PSUM") as psPV:
                for h in range(HPC):
                    h0 = h * D
                    for ir in range(T // IR):
                        i0 = ir * IR
                        nj = (MXL + i0 + IR) // P  # visible j-tiles
                        er_sb = erp.tile([P, NJT, IR], bf16, tag="er")
                        nc.sync.dma_start(
                            er_sb[:],
                            erel.ap()[h][:, i0:i0 + IR]
                                .rearrange("(a p) i -> p a i", p=P))
                        for b in range(B):
                            pvps = psPV.tile([P, IR], fp32, tag="pv")
                            for jt in range(nj):
                                sps = psS.tile([P, IR], fp32, tag="s")
                                nc.tensor.matmul(
                                    sps[:],
                                    kTb[h0:h0 + D, b, jt * P:(jt + 1) * P],
                                    qT[h0:h0 + D, b, i0:i0 + IR],
                                    start=True, stop=True)
                                exs = exsp.tile([P, IR], bf16, tag="exs")
                                nc.scalar.activation(exs[:], sps[:], AF.Exp,
                                                     scale=SCALE)
                                pex = pexp.tile([P, IR], bf16, tag="pex")
                                nc.vector.tensor_mul(
                                    pex[:], exs[:], er_sb[:, jt, :])
                                nc.tensor.matmul(
                                    pvps[:],
                                    vpv[:, b, jt, h * PVW:(h + 1) * PVW],
                                    pex[:],
                                    start=(jt == 0), stop=(jt == nj - 1))
                            rb = smp.tile([D, IR], fp32, tag="rb")
                            nc.vector.reciprocal(rb[:], pvps[D:2 * D, :])
                            nc.vector.tensor_mul(
                                qkvn[h0:h0 + D, b, i0:i0 + IR],
                                pvps[0:D, :], rb[:])

            # ---- Phase 3: output projection (partial) ----------------------
            with tc.tile_pool(name="psO", bufs=2, space="PSUM") as psO, \
                 tc.tile_pool(name="osb", bufs=3) as osbp:
                for b in range(B):
                    for tt in range(T // P):
                        t0 = tt * P
                        for eh in range(E // 512):
                            ops = psO.tile([P, 512], fp32, tag="o")
                            nc.tensor.matmul(
                                ops[:], qkvn[:, b, t0:t0 + P],
                                wo_sb[:, eh * 512:(eh + 1) * 512],
                                start=True, stop=True)
                            osb = osbp.tile([P, 512], fp32, tag="osb")
                            nc.vector.tensor_copy(osb[:], ops[:])
                            nc.sync.dma_start(
                                outp.ap()[b * T + t0:b * T + t0 + P,
                                          eh * 512:(eh + 1) * 512],
                                osb[:])

    nc.compile()
    return nc


def _get_program():
    if "nc" not in _CACHE:
        _CACHE["nc"] = _build_program()
    return _CACHE["nc"]


def _prep_inputs(x, xl, rel, Wq, bq, Wk, bk, Wv, bv, Wo):
    """Host-side sharding/layout prep. Returns per-core input maps."""
    import ml_dtypes

    bf = ml_dtypes.bfloat16
    xT = np.ascontiguousarray(x.reshape(BT, E).T)          # [E, BT]

    # mask (j >= i + MXL + 1) and rel bias folded into exp(rel * SCALE)
    jj = np.arange(J, dtype=np.int64)[:, None]
    ii = np.arange(T, dtype=np.int64)[None, :]
    maskT = jj >= (ii + MXL + 1)                           # [J, T]

    in_maps = []
    for c in range(NCORES):
        cs = slice(c * HDC, (c + 1) * HDC)
        relc = np.exp(rel[c * HPC:(c + 1) * HPC].transpose(0, 2, 1) * SCALE)
        relc[:, maskT] = 0.0
        in_maps.append({
            "xT": xT,
            "wq": np.ascontiguousarray(Wq[:, cs] * SCALE),
            "wk": np.ascontiguousarray(Wk[:, cs]),
            "wv": np.ascontiguousarray(Wv[:, cs]),
            "wo": np.ascontiguousarray(Wo[cs, :]).astype(bf),
            "bqd": np.ascontiguousarray(
                (bq[cs] * SCALE).reshape(HDC, 1).astype(np.float32)),
            "bkd": np.ascontiguousarray(bk[cs].reshape(HDC, 1)),
            "bvd": np.ascontiguousarray(bv[cs].reshape(1, HDC)),
            "kxlT": np.ascontiguousarray(xl[:, :, 0, cs].reshape(BT, HDC).T),
            "vxl": np.ascontiguousarray(
                xl[:, :, 1, cs].reshape(BT, HDC)).astype(bf),
            "erel": np.ascontiguousarray(relc).astype(bf),
        })
    return in_maps


def _run(inputs, trace=False, tmpdir=None, trace_cores=None):
    from concourse.bass_utils import run_bass_kernel_spmd

    f = lambda k: np.asarray(inputs[k], np.float32)
    in_maps = _prep_inputs(f("x"), f("xl_memory"), f("relative_positions"),
                           f("Wq"), f("bq"), f("Wk"), f("bk"),
                           f("Wv"), f("bv"), f("Wo"))
    bo = f("bo")

    nc = _get_program()
    kw = {}
    if trace:
        kw.update(trace=True, tmpdir=tmpdir, trace_cores=trace_cores)
    res = run_bass_kernel_spmd(nc, in_maps, list(range(NCORES)), **kw)

    out = np.zeros((BT, E), np.float32)
    kv = np.empty((B, T, 2, H * D), np.float32)
    for c in range(NCORES):
        cs = slice(c * HDC, (c + 1) * HDC)
        out += res.results[c]["outp"]
        kv[:, :, :, cs] = res.results[c]["kvp"].reshape(B, T, 2, HDC)
    out = out.reshape(B, T, E) + bo
    return (out, kv), res


def kernel(**inputs):
    outs, _ = _run(inputs, trace=False)
    return outs


# revision 12
# speedup vs baseline: 1.5425x; 1.2688x over previous
# Trainium2 Bass kernel for nn_CausalSelfAttention_58239756533763.
#
# Sharding: tensor-parallel over heads. 16 heads / 8 cores = 2 heads per
# core. Each core computes q/k/v projections for its 2 heads (all 4
# batches), attention with XL memory, and a partial output projection
# against its 128 rows of Wo. The host sums the 8 partial outputs (the
# "all-reduce" of the sharding hint, done at unshard time) and concatenates
# the per-core k/v slices into the kv_to_add_xl output.
#
# On-chip layout: activations flow in the transposed ("feature on
# partitions") layout so only k/v need on-device 128x128 PE transposes
# (for the kv output / P@V operand):
#   x^T, q^T, k^T, v^T are [feature, token]; scores are S^T = [j, i].
#   Softmax denominators come from a 64-wide ones block appended to v in
#   the P@V matmul, so the sums land on partitions 64:128 of the same
#   PSUM tile (vectorized reciprocal, no cross-partition reduce).
#   exp((S + rel) * s) = exp(S * s) * exp(rel * s): the host precomputes
#   exp(rel^T * s) in fp16 with masked (j, i) entries set to 0, so the
#   bias-add becomes an fp16 multiply and causal masking is exact.
# Matmuls run in fp16 (one PE pass, ~5e-4 quantization) with fp32 PSUM
# accumulation; softmax exp runs on the Scalar engine in fp32.

import numpy as np

B, T, MXL, E, H, D = 4, 1024, 1024, 1024, 16, 64
SCALE = D ** (-0.5)
NCORES = 8
HPC = H // NCORES          # heads per core = 2
HDC = HPC * D              # head-dim columns per core = 128
BT = B * T                 # 4096 tokens
J = MXL + T                # 2048 key positions
P = 128
IR = 512                   # i-range (query block, matmul free dim)
NJT = J // P               # 16 j-tiles
PVW = 2 * D                # per-head lhsT width in P@V: [v (64) | ones (64)]
ET = E // P                # 8 contraction tiles

_CACHE = {}


def _build_program():
    import concourse.mybir as mybir
    import concourse.tile as tile
    from concourse import bacc
    from concourse.masks import make_identity

    fp32 = mybir.dt.float32
    fp16 = mybir.dt.float16
    AF = mybir.ActivationFunctionType

    nc = bacc.Bacc("TRN2", target_bir_lowering=False, debug=False,
                   num_devices=NCORES)

    xTd = nc.dram_tensor("xTd", [E, BT], fp16, kind="ExternalInput")
    wq = nc.dram_tensor("wq", [E, HDC], fp16, kind="ExternalInput")
    wk = nc.dram_tensor("wk", [E, HDC], fp16, kind="ExternalInput")
    wv = nc.dram_tensor("wv", [E, HDC], fp16, kind="ExternalInput")
    wo = nc.dram_tensor("wo", [HDC, E], fp16, kind="ExternalInput")
    bqd = nc.dram_tensor("bqd", [HDC, 1], fp32, kind="ExternalInput")
    bkd = nc.dram_tensor("bkd", [HDC, 1], fp32, kind="ExternalInput")
    bvd = nc.dram_tensor("bvd", [HDC, 1], fp32, kind="ExternalInput")
    kxlT = nc.dram_tensor("kxlT", [HDC, BT], fp16, kind="ExternalInput")
    vxl = nc.dram_tensor("vxl", [BT, HDC], fp16, kind="ExternalInput")
    erel = nc.dram_tensor("erel", [HPC, J, T], fp16, kind="ExternalInput")
    outp = nc.dram_tensor("outp", [BT, E], fp32, kind="ExternalOutput")
    kvp = nc.dram_tensor("kvp", [BT, 2, HDC], fp32, kind="ExternalOutput")

    with tile.TileContext(nc) as tc:
        with tc.tile_pool(name="const", bufs=1) as constp, \
             tc.tile_pool(name="big", bufs=1) as bigp:
            ident = constp.tile([P, P], fp16)
            make_identity(nc, ident[:])
            bq_sb = constp.tile([HDC, 1], fp32)
            bk_sb = constp.tile([HDC, 1], fp32)
            bv_sb = constp.tile([HDC, 1], fp32)
            wo_sb = constp.tile([HDC, E], fp16)

            kT = [bigp.tile([P, J], fp16, tag=f"kT{b}", name=f"kT{b}")
                  for b in range(B)]
            vpv = [bigp.tile([P, NJT, HPC * PVW], fp16, tag=f"vpv{b}",
                              name=f"vpv{b}") for b in range(B)]
            qT = [bigp.tile([P, T], fp16, tag=f"qT{b}", name=f"qT{b}")
                  for b in range(B)]
            qkvn = [bigp.tile([P, T], fp16, tag=f"qkvn{b}",
                               name=f"qkvn{b}") for b in range(B)]
            er_sb = bigp.tile([P, HPC, NJT, T], fp16)

            # ---- Phase 1: projections --------------------------------------
            with tc.tile_pool(name="w3", bufs=1) as w3p, \
                 tc.tile_pool(name="xt", bufs=3) as xtp, \
                 tc.tile_pool(name="ps1", bufs=2, space="PSUM") as ps1, \
                 tc.tile_pool(name="sb1", bufs=3) as sb1:
                wq_sb = w3p.tile([P, ET, HDC], fp16)
                wk_sb = w3p.tile([P, ET, HDC], fp16)
                wv_sb = w3p.tile([P, ET, HDC], fp16)
                nc.sync.dma_start(
                    wq_sb[:], wq.ap().rearrange("(a p) m -> p a m", p=P))
                nc.sync.dma_start(
                    wk_sb[:], wk.ap().rearrange("(a p) m -> p a m", p=P))
                nc.sync.dma_start(
                    wv_sb[:], wv.ap().rearrange("(a p) m -> p a m", p=P))
                nc.sync.dma_start(bq_sb[:], bqd.ap())
                nc.sync.dma_start(bk_sb[:], bkd.ap())
                nc.sync.dma_start(bv_sb[:], bvd.ap())

                for b in range(B):
                    for tr in range(T // IR):
                        col0 = b * T + tr * IR
                        xt = xtp.tile([P, ET, IR], fp16)
                        nc.sync.dma_start(
                            xt[:],
                            xTd.ap()[:, col0:col0 + IR]
                               .rearrange("(a p) t -> p a t", p=P))
                        qps = ps1.tile([P, IR], fp32, tag="qps")
                        for e in range(ET):
                            nc.tensor.matmul(qps[:], wq_sb[:, e, :],
                                             xt[:, e, :], start=(e == 0),
                                             stop=(e == ET - 1))
                        nc.scalar.activation(
                            qT[b][:, tr * IR:(tr + 1) * IR], qps[:],
                            AF.Identity, bias=bq_sb[:])
                        kps = ps1.tile([P, IR], fp32, tag="kps")
                        for e in range(ET):
                            nc.tensor.matmul(kps[:], wk_sb[:, e, :],
                                             xt[:, e, :], start=(e == 0),
                                             stop=(e == ET - 1))
                        nc.scalar.activation(
                            kT[b][:, MXL + tr * IR:MXL + (tr + 1) * IR],
                            kps[:], AF.Identity, bias=bk_sb[:])
                        vps = ps1.tile([P, IR], fp32, tag="vps")
                        for e in range(ET):
                            nc.tensor.matmul(vps[:], wv_sb[:, e, :],
                                             xt[:, e, :], start=(e == 0),
                                             stop=(e == ET - 1))
                        vTs = sb1.tile([P, IR], fp16, tag="vTs")
                        nc.scalar.activation(vTs[:], vps[:],
                                             AF.Identity, bias=bv_sb[:])

                        for s in range(IR // P):
                            t0 = tr * IR + s * P
                            jt = MXL // P + t0 // P
                            # v natural via PE transpose; fp32 copy to kv out
                            vtp = ps1.tile([P, P], fp16, tag="tp")
                            nc.tensor.transpose(vtp[:],
                                                vTs[:, s * P:(s + 1) * P],
                                                ident[:])
                            vsb = sb1.tile([P, P], fp32, tag="vsb")
                            nc.scalar.copy(vsb[:], vtp[:])
                            nc.vector.tensor_copy(vpv[b][:, jt, 0:D],
                                                  vsb[:, 0:D])
                            nc.vector.tensor_copy(vpv[b][:, jt, PVW:PVW + D],
                                                  vsb[:, D:2 * D])
                            nc.sync.dma_start(
                                kvp.ap()[b * T + t0:b * T + t0 + P, 1, :],
                                vsb[:])
                            # k natural (current chunk) for the kv output
                            ktp = ps1.tile([P, P], fp16, tag="tp")
                            nc.tensor.transpose(
                                ktp[:],
                                kT[b][:, MXL + t0:MXL + t0 + P],
                                ident[:])
                            ksb = sb1.tile([P, P], fp32, tag="ksb")
                            nc.vector.tensor_copy(ksb[:], ktp[:])
                            nc.sync.dma_start(
                                kvp.ap()[b * T + t0:b * T + t0 + P, 0, :],
                                ksb[:])
                    if b == 0:
                        # phase-2 feed DMAs: emitted after b=0 so the first
                        # projection matmuls aren't starved by them
                        nc.sync.dma_start(wo_sb[:], wo.ap())
                        kxl_r = kxlT.ap().rearrange("p (b t) -> p b t", b=B)
                        vxl_r = vxl.ap().rearrange("(b j p) c -> p b j c",
                                                   b=B, p=P)
                        for bb in range(B):
                            nc.sync.dma_start(kT[bb][:, 0:MXL],
                                              kxl_r[:, bb, :])
                            nc.sync.dma_start(vpv[bb][:, 0:MXL // P, 0:D],
                                              vxl_r[:, bb, :, 0:D])
                            nc.sync.dma_start(
                                vpv[bb][:, 0:MXL // P, PVW:PVW + D],
                                vxl_r[:, bb, :, D:2 * D])
                            nc.vector.memset(vpv[bb][:, :, D:PVW], 1.0)
                            nc.vector.memset(vpv[bb][:, :, PVW + D:2 * PVW],
                                             1.0)
                        for h in range(HPC):
                            nc.sync.dma_start(
                                er_sb[:, h],
                                erel.ap()[h].rearrange("(a p) i -> p a i",
                                                       p=P))

            # ---- Phase 2: attention, with interleaved output projection ----
            with tc.tile_pool(name="exs", bufs=3) as exsp, \
                 tc.tile_pool(name="pex", bufs=3) as pexp, \
                 tc.tile_pool(name="sm", bufs=2) as smp, \
                 tc.tile_pool(name="osb", bufs=3) as osbp, \
                 tc.tile_pool(name="psS", bufs=2, space="PSUM") as psS, \
                 tc.tile_pool(name="psPV", bufs=2, space="PSUM") as psPV, \
                 tc.tile_pool(name="psO", bufs=2, space="PSUM") as psO:
                for ir in range(T // IR):
                    i0 = ir * IR
                    nj = (MXL + i0 + IR) // P  # visible j-tiles (12 or 16)
                    for b in range(B):
                        for h in range(HPC):
                            h0 = h * D
                            pvps = psPV.tile([P, IR], fp32, tag="pv")
                            for jp in range(nj // 2):
                                sps = psS.tile([P, 2, IR], fp32, tag="s")
                                for u in range(2):
                                    jt = jp * 2 + u
                                    nc.tensor.matmul(
                                        sps[:, u, :],
                                        kT[b][h0:h0 + D, jt * P:(jt + 1) * P],
                                        qT[b][h0:h0 + D, i0:i0 + IR],
                                        start=True, stop=True)
                                exs = exsp.tile([P, 2, IR], fp16, tag="exs")
                                nc.scalar.activation(exs[:], sps[:], AF.Exp,
                                                     scale=SCALE)
                                pex = pexp.tile([P, 2, IR], fp16, tag="pex")
                                nc.vector.tensor_mul(
                                    pex[:], exs[:],
                                    er_sb[:, h, jp * 2:jp * 2 + 2,
                                          i0:i0 + IR])
                                for u in range(2):
                                    jt = jp * 2 + u
                                    nc.tensor.matmul(
                                        pvps[:],
                                        vpv[b][:, jt,
                                               h * PVW:(h + 1) * PVW],
                                        pex[:, u, :],
                                        start=(jt == 0), stop=(jt == nj - 1))
                            rb = smp.tile([D, IR], fp32, tag="rb")
                            nc.vector.reciprocal(rb[:], pvps[D:2 * D, :])
                            nc.vector.tensor_mul(
                                qkvn[b][h0:h0 + D, i0:i0 + IR],
                                pvps[0:D, :], rb[:])
                        # partial output projection for this (b, i-range)
                        for tt in range(IR // P):
                            t0 = i0 + tt * P
                            for eh in range(E // 512):
                                ops = psO.tile([P, 512], fp32, tag="o")
                                nc.tensor.matmul(
                                    ops[:], qkvn[b][:, t0:t0 + P],
                                    wo_sb[:, eh * 512:(eh + 1) * 512],
                                    start=True, stop=True)
                                osb = osbp.tile([P, 512], fp32, tag="osb")
                                nc.vector.tensor_copy(osb[:], ops[:])
                                nc.sync.dma_start(
                                    outp.ap()[b * T + t0:b * T + t0 + P,
                                              eh * 512:(eh + 1) * 512],
                                    osb[:])

    nc.compile()
    return nc


def _get_program():
    if "nc" not in _CACHE:
        _CACHE["nc"] = _build_program()
    return _CACHE["nc"]


def _prep_inputs(x, xl, rel, Wq, bq, Wk, bk, Wv, bv, Wo):
    """Host-side sharding/layout prep. Returns per-core input maps."""
    f16 = np.float16
    xT = np.ascontiguousarray(x.reshape(BT, E).T).astype(f16)   # [E, BT]

    # mask (j >= i + MXL + 1) and rel bias folded into exp(rel * SCALE)
    jj = np.arange(J, dtype=np.int64)[:, None]
    ii = np.arange(T, dtype=np.int64)[None, :]
    maskT = jj >= (ii + MXL + 1)                                # [J, T]

    in_maps = []
    for c in range(NCORES):
        cs = slice(c * HDC, (c + 1) * HDC)
        relc = np.exp(rel[c * HPC:(c + 1) * HPC].transpose(0, 2, 1) * SCALE)
        relc[:, maskT] = 0.0
        in_maps.append({
            "xTd": xT,
            "wq": np.ascontiguousarray(Wq[:, cs] * SCALE).astype(f16),
            "wk": np.ascontiguousarray(Wk[:, cs]).astype(f16),
            "wv": np.ascontiguousarray(Wv[:, cs]).astype(f16),
            "wo": np.ascontiguousarray(Wo[cs, :]).astype(f16),
            "bqd": np.ascontiguousarray(
                (bq[cs] * SCALE).reshape(HDC, 1).astype(np.float32)),
            "bkd": np.ascontiguousarray(bk[cs].reshape(HDC, 1)),
            "bvd": np.ascontiguousarray(bv[cs].reshape(HDC, 1)),
            "kxlT": np.ascontiguousarray(
                xl[:, :, 0, cs].reshape(BT, HDC).T).astype(f16),
            "vxl": np.ascontiguousarray(
                xl[:, :, 1, cs].reshape(BT, HDC)).astype(f16),
            "erel": np.ascontiguousarray(relc).astype(f16),
        })
    return in_maps


def _run(inputs, trace=False, tmpdir=None, trace_cores=None):
    from concourse.bass_utils import run_bass_kernel_spmd

    f = lambda k: np.asarray(inputs[k], np.float32)
    in_maps = _prep_inputs(f("x"), f("xl_memory"), f("relative_positions"),
                           f("Wq"), f("bq"), f("Wk"), f("bk"),
                           f("Wv"), f("bv"), f("Wo"))
    bo = f("bo")

    nc = _get_program()
    kw = {}
    if trace:
        kw.update(trace=True, tmpdir=tmpdir, trace_cores=trace_cores)
    res = run_bass_kernel_spmd(nc, in_maps, list(range(NCORES)), **kw)

    out = np.zeros((BT, E), np.float32)
    kv = np.empty((B, T, 2, H * D), np.float32)
    for c in range(NCORES):
        cs = slice(c * HDC, (c + 1) * HDC)
        out += res.results[c]["outp"]
        kv[:, :, :, cs] = res.results[c]["kvp"].reshape(B, T, 2, HDC)
    out = out.reshape(B, T, E) + bo
    return (out, kv), res


def kernel(**inputs):
    outs, _ = _run(inputs, trace=False)
    return outs


# revision 13
# speedup vs baseline: 1.8235x; 1.1822x over previous
# Trainium2 Bass kernel for nn_CausalSelfAttention_58239756533763.
#
# Sharding: tensor-parallel over heads. 16 heads / 8 cores = 2 heads per
# core. Each core computes q/k/v projections for its 2 heads (all 4
# batches), attention with XL memory, and a partial output projection
# against its 128 rows of Wo. The host sums the 8 partial outputs (the
# "all-reduce" of the sharding hint, done at unshard time) and concatenates
# the per-core k/v slices into the kv_to_add_xl output.
#
# On-chip layout: activations flow in the transposed ("feature on
# partitions") layout so only k/v need on-device 128x128 PE transposes
# (for the kv output / P@V operand):
#   x^T, q^T, k^T, v^T are [feature, token]; scores are S^T = [j, i].
#   Softmax denominators come from a 64-wide ones block appended to v in
#   the P@V matmul, so the sums land on partitions 64:128 of the same
#   PSUM tile (vectorized reciprocal, no cross-partition reduce).
#   exp((S + rel) * s) = exp(S * s) * exp(rel * s): the host precomputes
#   exp(rel^T * s) in fp16 with masked (j, i) entries set to 0, so the
#   bias-add becomes an fp16 multiply and causal masking is exact.
# Matmuls run in fp16 (one PE pass, ~5e-4 quantization) with fp32 PSUM
# accumulation; softmax exp runs on the Scalar engine in fp32.

import numpy as np

B, T, MXL, E, H, D = 4, 1024, 1024, 1024, 16, 64
SCALE = D ** (-0.5)
NCORES = 8
HPC = H // NCORES          # heads per core = 2
HDC = HPC * D              # head-dim columns per core = 128
BT = B * T                 # 4096 tokens
J = MXL + T                # 2048 key positions
P = 128
IR = 512                   # i-range (query block, matmul free dim)
NJT = J // P               # 16 j-tiles
PVW = 2 * D                # per-head lhsT width in P@V: [v (64) | ones (64)]
ET = E // P                # 8 contraction tiles

_CACHE = {}


def _build_program():
    import concourse.mybir as mybir
    import concourse.tile as tile
    from concourse import bacc
    from concourse.masks import make_identity

    fp32 = mybir.dt.float32
    fp16 = mybir.dt.float16
    AF = mybir.ActivationFunctionType

    nc = bacc.Bacc("TRN2", target_bir_lowering=False, debug=False,
                   num_devices=NCORES)

    xTd = nc.dram_tensor("xTd", [E, BT], fp16, kind="ExternalInput")
    wq = nc.dram_tensor("wq", [E, HDC], fp16, kind="ExternalInput")
    wk = nc.dram_tensor("wk", [E, HDC], fp16, kind="ExternalInput")
    wv = nc.dram_tensor("wv", [E, HDC], fp16, kind="ExternalInput")
    wo = nc.dram_tensor("wo", [HDC, E], fp16, kind="ExternalInput")
    bqd = nc.dram_tensor("bqd", [HDC, 1], fp32, kind="ExternalInput")
    bkd = nc.dram_tensor("bkd", [HDC, 1], fp32, kind="ExternalInput")
    bvd = nc.dram_tensor("bvd", [HDC, 1], fp32, kind="ExternalInput")
    kxlT = nc.dram_tensor("kxlT", [HDC, BT], fp16, kind="ExternalInput")
    vxl = nc.dram_tensor("vxl", [BT, HDC], fp16, kind="ExternalInput")
    erel = nc.dram_tensor("erel", [HPC, J, T], fp16, kind="ExternalInput")
    outp = nc.dram_tensor("outp", [BT, E], fp32, kind="ExternalOutput")
    kvp = nc.dram_tensor("kvp", [BT, 2, HDC], fp32, kind="ExternalOutput")

    with tile.TileContext(nc) as tc:
        with tc.tile_pool(name="const", bufs=1) as constp, \
             tc.tile_pool(name="big", bufs=1) as bigp:
            ident = constp.tile([P, P], fp16)
            make_identity(nc, ident[:])
            bq_sb = constp.tile([HDC, 1], fp32)
            bk_sb = constp.tile([HDC, 1], fp32)
            bv_sb = constp.tile([HDC, 1], fp32)
            wo_sb = constp.tile([HDC, E], fp16)

            kT = [bigp.tile([P, J], fp16, tag=f"kT{b}", name=f"kT{b}")
                  for b in range(B)]
            vpv = [bigp.tile([P, NJT, HPC * PVW], fp16, tag=f"vpv{b}",
                              name=f"vpv{b}") for b in range(B)]
            qT = [bigp.tile([P, T], fp16, tag=f"qT{b}", name=f"qT{b}")
                  for b in range(B)]
            qkvn = [bigp.tile([P, T], fp16, tag=f"qkvn{b}",
                               name=f"qkvn{b}") for b in range(B)]
            er_sb = bigp.tile([P, HPC, NJT, T], fp16)

            # ---- Phase 1: projections --------------------------------------
            with tc.tile_pool(name="w3", bufs=1) as w3p, \
                 tc.tile_pool(name="xt", bufs=3) as xtp, \
                 tc.tile_pool(name="ps1", bufs=2, space="PSUM") as ps1, \
                 tc.tile_pool(name="sb1", bufs=3) as sb1:
                wq_sb = w3p.tile([P, ET, HDC], fp16)
                wk_sb = w3p.tile([P, ET, HDC], fp16)
                wv_sb = w3p.tile([P, ET, HDC], fp16)
                nc.sync.dma_start(
                    wq_sb[:], wq.ap().rearrange("(a p) m -> p a m", p=P))
                nc.sync.dma_start(
                    wk_sb[:], wk.ap().rearrange("(a p) m -> p a m", p=P))
                nc.sync.dma_start(
                    wv_sb[:], wv.ap().rearrange("(a p) m -> p a m", p=P))
                nc.sync.dma_start(bq_sb[:], bqd.ap())
                nc.sync.dma_start(bk_sb[:], bkd.ap())
                nc.sync.dma_start(bv_sb[:], bvd.ap())

                for b in range(B):
                    for tr in range(T // IR):
                        col0 = b * T + tr * IR
                        xt = xtp.tile([P, ET, IR], fp16)
                        nc.sync.dma_start(
                            xt[:],
                            xTd.ap()[:, col0:col0 + IR]
                               .rearrange("(a p) t -> p a t", p=P))
                        qps = ps1.tile([P, IR], fp32, tag="qps")
                        for e in range(ET):
                            nc.tensor.matmul(qps[:], wq_sb[:, e, :],
                                             xt[:, e, :], start=(e == 0),
                                             stop=(e == ET - 1))
                        nc.scalar.activation(
                            qT[b][:, tr * IR:(tr + 1) * IR], qps[:],
                            AF.Identity, bias=bq_sb[:])
                        kps = ps1.tile([P, IR], fp32, tag="kps")
                        for e in range(ET):
                            nc.tensor.matmul(kps[:], wk_sb[:, e, :],
                                             xt[:, e, :], start=(e == 0),
                                             stop=(e == ET - 1))
                        nc.scalar.activation(
                            kT[b][:, MXL + tr * IR:MXL + (tr + 1) * IR],
                            kps[:], AF.Identity, bias=bk_sb[:])
                        vps = ps1.tile([P, IR], fp32, tag="vps")
                        for e in range(ET):
                            nc.tensor.matmul(vps[:], wv_sb[:, e, :],
                                             xt[:, e, :], start=(e == 0),
                                             stop=(e == ET - 1))
                        vTs = sb1.tile([P, IR], fp16, tag="vTs")
                        nc.scalar.activation(vTs[:], vps[:],
                                             AF.Identity, bias=bv_sb[:])

                        for s in range(IR // P):
                            t0 = tr * IR + s * P
                            jt = MXL // P + t0 // P
                            # v natural via PE transpose; fp32 copy to kv out
                            vtp = ps1.tile([P, P], fp16, tag="tp")
                            nc.tensor.transpose(vtp[:],
                                                vTs[:, s * P:(s + 1) * P],
                                                ident[:])
                            vsb = sb1.tile([P, P], fp32, tag="vsb")
                            nc.scalar.copy(vsb[:], vtp[:])
                            nc.vector.tensor_copy(vpv[b][:, jt, 0:D],
                                                  vsb[:, 0:D])
                            nc.vector.tensor_copy(vpv[b][:, jt, PVW:PVW + D],
                                                  vsb[:, D:2 * D])
                            nc.sync.dma_start(
                                kvp.ap()[b * T + t0:b * T + t0 + P, 1, :],
                                vsb[:])
                            # k natural (current chunk) for the kv output
                            ktp = ps1.tile([P, P], fp16, tag="tp")
                            nc.tensor.transpose(
                                ktp[:],
                                kT[b][:, MXL + t0:MXL + t0 + P],
                                ident[:])
                            ksb = sb1.tile([P, P], fp32, tag="ksb")
                            nc.vector.tensor_copy(ksb[:], ktp[:])
                            nc.sync.dma_start(
                                kvp.ap()[b * T + t0:b * T + t0 + P, 0, :],
                                ksb[:])
                    if b == 0:
                        # phase-2 feed DMAs: emitted after b=0 so the first
                        # projection matmuls aren't starved by them
                        nc.sync.dma_start(wo_sb[:], wo.ap())
                        kxl_r = kxlT.ap().rearrange("p (b t) -> p b t", b=B)
                        vxl_r = vxl.ap().rearrange("(b j p) c -> p b j c",
                                                   b=B, p=P)
                        for bb in range(B):
                            nc.sync.dma_start(kT[bb][:, 0:MXL],
                                              kxl_r[:, bb, :])
                            nc.sync.dma_start(vpv[bb][:, 0:MXL // P, 0:D],
                                              vxl_r[:, bb, :, 0:D])
                            nc.sync.dma_start(
                                vpv[bb][:, 0:MXL // P, PVW:PVW + D],
                                vxl_r[:, bb, :, D:2 * D])
                            nc.vector.memset(vpv[bb][:, :, D:PVW], 1.0)
                            nc.vector.memset(vpv[bb][:, :, PVW + D:2 * PVW],
                                             1.0)
                        for h in range(HPC):
                            nc.sync.dma_start(
                                er_sb[:, h],
                                erel.ap()[h].rearrange("(a p) i -> p a i",
                                                       p=P))

            # ---- Phase 2: attention, with interleaved output projection ----
            with tc.tile_pool(name="exs", bufs=3) as exsp, \
                 tc.tile_pool(name="pex", bufs=3) as pexp, \
                 tc.tile_pool(name="sm", bufs=2) as smp, \
                 tc.tile_pool(name="osb", bufs=3) as osbp, \
                 tc.tile_pool(name="psS", bufs=2, space="PSUM") as psS, \
                 tc.tile_pool(name="psPV", bufs=2, space="PSUM") as psPV, \
                 tc.tile_pool(name="psO", bufs=2, space="PSUM") as psO:
                for ir in range(T // IR):
                    i0 = ir * IR
                    nj = (MXL + i0 + IR) // P  # visible j-tiles (12 or 16)
                    for b in range(B):
                        for h in range(HPC):
                            h0 = h * D
                            pvps = psPV.tile([P, IR], fp32, tag="pv")
                            for jp in range(nj // 2):
                                sps = psS.tile([P, 2, IR], fp32, tag="s")
                                for u in range(2):
                                    jt = jp * 2 + u
                                    nc.tensor.matmul(
                                        sps[:, u, :],
                                        kT[b][h0:h0 + D, jt * P:(jt + 1) * P],
                                        qT[b][h0:h0 + D, i0:i0 + IR],
                                        start=True, stop=True)
                                exs = exsp.tile([P, 2, IR], fp16, tag="exs")
                                nc.scalar.activation(exs[:], sps[:], AF.Exp,
                                                     scale=SCALE)
                                pex = pexp.tile([P, 2, IR], fp16, tag="pex")
                                nc.vector.tensor_mul(
                                    pex[:], exs[:],
                                    er_sb[:, h, jp * 2:jp * 2 + 2,
                                          i0:i0 + IR])
                                for u in range(2):
                                    jt = jp * 2 + u
                                    nc.tensor.matmul(
                                        pvps[:],
                                        vpv[b][:, jt,
                                               h * PVW:(h + 1) * PVW],
                                        pex[:, u, :],
                                        start=(jt == 0), stop=(jt == nj - 1))
                            rs = smp.tile([D, IR], fp32, tag="rs")
                            nc.vector.tensor_copy(rs[:], pvps[D:2 * D, :])
                            rb = smp.tile([D, IR], fp32, tag="rb")
                            nc.vector.reciprocal_approx_fast(rb[:], rs[:])
                            nc.vector.tensor_mul(
                                qkvn[b][h0:h0 + D, i0:i0 + IR],
                                pvps[0:D, :], rb[:])
                        # partial output projection for this (b, i-range)
                        for tt in range(IR // P):
                            t0 = i0 + tt * P
                            for eh in range(E // 512):
                                ops = psO.tile([P, 512], fp32, tag="o")
                                nc.tensor.matmul(
                                    ops[:], qkvn[b][:, t0:t0 + P],
                                    wo_sb[:, eh * 512:(eh + 1) * 512],
                                    start=True, stop=True)
                                osb = osbp.tile([P, 512], fp32, tag="osb")
                                nc.vector.tensor_copy(osb[:], ops[:])
                                nc.sync.dma_start(
                                    outp.ap()[b * T + t0:b * T + t0 + P,
                                              eh * 512:(eh + 1) * 512],
                                    osb[:])

    nc.compile()
    return nc


def _get_program():
    if "nc" not in _CACHE:
        _CACHE["nc"] = _build_program()
    return _CACHE["nc"]


def _prep_inputs(x, xl, rel, Wq, bq, Wk, bk, Wv, bv, Wo):
    """Host-side sharding/layout prep. Returns per-core input maps."""
    f16 = np.float16
    xT = np.ascontiguousarray(x.reshape(BT, E).T).astype(f16)   # [E, BT]

    # mask (j >= i + MXL + 1) and rel bias folded into exp(rel * SCALE)
    jj = np.arange(J, dtype=np.int64)[:, None]
    ii = np.arange(T, dtype=np.int64)[None, :]
    maskT = jj >= (ii + MXL + 1)                                # [J, T]

    in_maps = []
    for c in range(NCORES):
        cs = slice(c * HDC, (c + 1) * HDC)
        relc = np.exp(rel[c * HPC:(c + 1) * HPC].transpose(0, 2, 1) * SCALE)
        relc[:, maskT] = 0.0
        in_maps.append({
            "xTd": xT,
            "wq": np.ascontiguousarray(Wq[:, cs] * SCALE).astype(f16),
            "wk": np.ascontiguousarray(Wk[:, cs]).astype(f16),
            "wv": np.ascontiguousarray(Wv[:, cs]).astype(f16),
            "wo": np.ascontiguousarray(Wo[cs, :]).astype(f16),
            "bqd": np.ascontiguousarray(
                (bq[cs] * SCALE).reshape(HDC, 1).astype(np.float32)),
            "bkd": np.ascontiguousarray(bk[cs].reshape(HDC, 1)),
            "bvd": np.ascontiguousarray(bv[cs].reshape(HDC, 1)),
            "kxlT": np.ascontiguousarray(
                xl[:, :, 0, cs].reshape(BT, HDC).T).astype(f16),
            "vxl": np.ascontiguousarray(
                xl[:, :, 1, cs].reshape(BT, HDC)).astype(f16),
            "erel": np.ascontiguousarray(relc).astype(f16),
        })
    return in_maps


def _run(inputs, trace=False, tmpdir=None, trace_cores=None):
    from concourse.bass_utils import run_bass_kernel_spmd

    f = lambda k: np.asarray(inputs[k], np.float32)
    in_maps = _prep_inputs(f("x"), f("xl_memory"), f("relative_positions"),
                           f("Wq"), f("bq"), f("Wk"), f("bk"),
                           f("Wv"), f("bv"), f("Wo"))
    bo = f("bo")

    nc = _get_program()
    kw = {}
    if trace:
        kw.update(trace=True, tmpdir=tmpdir, trace_cores=trace_cores)
    res = run_bass_kernel_spmd(nc, in_maps, list(range(NCORES)), **kw)

    out = np.zeros((BT, E), np.float32)
    kv = np.empty((B, T, 2, H * D), np.float32)
    for c in range(NCORES):
        cs = slice(c * HDC, (c + 1) * HDC)
        out += res.results[c]["outp"]
        kv[:, :, :, cs] = res.results[c]["kvp"].reshape(B, T, 2, HDC)
    out = out.reshape(B, T, E) + bo
    return (out, kv), res


def kernel(**inputs):
    outs, _ = _run(inputs, trace=False)
    return outs


# revision 14
# speedup vs baseline: 1.8438x; 1.0111x over previous
# Trainium2 Bass kernel for nn_CausalSelfAttention_58239756533763.
#
# Sharding: tensor-parallel over heads. 16 heads / 8 cores = 2 heads per
# core. Each core computes q/k/v projections for its 2 heads (all 4
# batches), attention with XL memory, and a partial output projection
# against its 128 rows of Wo. The host sums the 8 partial outputs (the
# "all-reduce" of the sharding hint, done at unshard time) and concatenates
# the per-core k/v slices into the kv_to_add_xl output.
#
# On-chip layout: activations flow in the transposed ("feature on
# partitions") layout so only k/v need on-device 128x128 PE transposes
# (for the kv output / P@V operand):
#   x^T, q^T, k^T, v^T are [feature, token]; scores are S^T = [j, i].
#   Softmax denominators come from a 64-wide ones block appended to v in
#   the P@V matmul, so the sums land on partitions 64:128 of the same
#   PSUM tile (vectorized reciprocal, no cross-partition reduce).
#   exp((S + rel) * s) = exp(S * s) * exp(rel * s): the host precomputes
#   exp(rel^T * s) in fp16 with masked (j, i) entries set to 0, so the
#   bias-add becomes an fp16 multiply and causal masking is exact.
# Matmuls run in fp16 (one PE pass, ~5e-4 quantization) with fp32 PSUM
# accumulation; softmax exp runs on the Scalar engine in fp32.

import numpy as np

B, T, MXL, E, H, D = 4, 1024, 1024, 1024, 16, 64
SCALE = D ** (-0.5)
NCORES = 8
HPC = H // NCORES          # heads per core = 2
HDC = HPC * D              # head-dim columns per core = 128
BT = B * T                 # 4096 tokens
J = MXL + T                # 2048 key positions
P = 128
IR = 512                   # i-range (query block, matmul free dim)
NJT = J // P               # 16 j-tiles
PVW = 2 * D                # per-head lhsT width in P@V: [v (64) | ones (64)]
ET = E // P                # 8 contraction tiles

_CACHE = {}


def _build_program():
    import concourse.mybir as mybir
    import concourse.tile as tile
    from concourse import bacc
    from concourse.masks import make_identity

    fp32 = mybir.dt.float32
    fp16 = mybir.dt.float16
    AF = mybir.ActivationFunctionType

    nc = bacc.Bacc("TRN2", target_bir_lowering=False, debug=False,
                   num_devices=NCORES)

    xTd = nc.dram_tensor("xTd", [E, BT], fp16, kind="ExternalInput")
    wq = nc.dram_tensor("wq", [E, HDC], fp16, kind="ExternalInput")
    wk = nc.dram_tensor("wk", [E, HDC], fp16, kind="ExternalInput")
    wv = nc.dram_tensor("wv", [E, HDC], fp16, kind="ExternalInput")
    wo = nc.dram_tensor("wo", [HDC, E], fp16, kind="ExternalInput")
    bqd = nc.dram_tensor("bqd", [HDC, 1], fp32, kind="ExternalInput")
    bkd = nc.dram_tensor("bkd", [HDC, 1], fp32, kind="ExternalInput")
    bvd = nc.dram_tensor("bvd", [HDC, 1], fp32, kind="ExternalInput")
    kxlT = nc.dram_tensor("kxlT", [HDC, BT], fp16, kind="ExternalInput")
    vxl = nc.dram_tensor("vxl", [BT, HDC], fp16, kind="ExternalInput")
    erel = nc.dram_tensor("erel", [HPC, J, T], fp16, kind="ExternalInput")
    outp = nc.dram_tensor("outp", [BT, E], fp32, kind="ExternalOutput")
    kvp = nc.dram_tensor("kvp", [BT, 2, HDC], fp32, kind="ExternalOutput")

    with tile.TileContext(nc) as tc:
        with tc.tile_pool(name="const", bufs=1) as constp, \
             tc.tile_pool(name="big", bufs=1) as bigp:
            ident = constp.tile([P, P], fp16)
            make_identity(nc, ident[:])
            bq_sb = constp.tile([HDC, 1], fp32)
            bk_sb = constp.tile([HDC, 1], fp32)
            bv_sb = constp.tile([HDC, 1], fp32)
            wo_sb = constp.tile([HDC, E], fp16)

            kT = [bigp.tile([P, J], fp16, tag=f"kT{b}", name=f"kT{b}")
                  for b in range(B)]
            vpv = [bigp.tile([P, NJT, HPC * PVW], fp16, tag=f"vpv{b}",
                              name=f"vpv{b}") for b in range(B)]
            qT = [bigp.tile([P, T], fp16, tag=f"qT{b}", name=f"qT{b}")
                  for b in range(B)]
            qkvn = [bigp.tile([P, T], fp16, tag=f"qkvn{b}",
                               name=f"qkvn{b}") for b in range(B)]
            er_sb = bigp.tile([P, HPC, NJT, T], fp16)

            # ---- Phase 1: projections --------------------------------------
            with tc.tile_pool(name="w3", bufs=1) as w3p, \
                 tc.tile_pool(name="xt", bufs=3) as xtp, \
                 tc.tile_pool(name="ps1", bufs=2, space="PSUM") as ps1, \
                 tc.tile_pool(name="sb1", bufs=3) as sb1:
                wq_sb = w3p.tile([P, ET, HDC], fp16)
                wk_sb = w3p.tile([P, ET, HDC], fp16)
                wv_sb = w3p.tile([P, ET, HDC], fp16)
                nc.sync.dma_start(
                    wq_sb[:], wq.ap().rearrange("(a p) m -> p a m", p=P))
                nc.sync.dma_start(
                    wk_sb[:], wk.ap().rearrange("(a p) m -> p a m", p=P))
                nc.sync.dma_start(
                    wv_sb[:], wv.ap().rearrange("(a p) m -> p a m", p=P))
                nc.sync.dma_start(bq_sb[:], bqd.ap())
                nc.sync.dma_start(bk_sb[:], bkd.ap())
                nc.sync.dma_start(bv_sb[:], bvd.ap())

                for b in range(B):
                    for tr in range(T // IR):
                        col0 = b * T + tr * IR
                        xt = xtp.tile([P, ET, IR], fp16)
                        nc.sync.dma_start(
                            xt[:],
                            xTd.ap()[:, col0:col0 + IR]
                               .rearrange("(a p) t -> p a t", p=P))
                        qps = ps1.tile([P, IR], fp32, tag="qps")
                        for e in range(ET):
                            nc.tensor.matmul(qps[:], wq_sb[:, e, :],
                                             xt[:, e, :], start=(e == 0),
                                             stop=(e == ET - 1))
                        nc.scalar.activation(
                            qT[b][:, tr * IR:(tr + 1) * IR], qps[:],
                            AF.Identity, bias=bq_sb[:])
                        kps = ps1.tile([P, IR], fp32, tag="kps")
                        for e in range(ET):
                            nc.tensor.matmul(kps[:], wk_sb[:, e, :],
                                             xt[:, e, :], start=(e == 0),
                                             stop=(e == ET - 1))
                        nc.scalar.activation(
                            kT[b][:, MXL + tr * IR:MXL + (tr + 1) * IR],
                            kps[:], AF.Identity, bias=bk_sb[:])
                        vps = ps1.tile([P, IR], fp32, tag="vps")
                        for e in range(ET):
                            nc.tensor.matmul(vps[:], wv_sb[:, e, :],
                                             xt[:, e, :], start=(e == 0),
                                             stop=(e == ET - 1))
                        vTs = sb1.tile([P, IR], fp16, tag="vTs")
                        nc.scalar.activation(vTs[:], vps[:],
                                             AF.Identity, bias=bv_sb[:])

                        for s in range(IR // P):
                            t0 = tr * IR + s * P
                            jt = MXL // P + t0 // P
                            # v natural via PE transpose; fp32 copy to kv out
                            vtp = ps1.tile([P, P], fp16, tag="tp")
                            nc.tensor.transpose(vtp[:],
                                                vTs[:, s * P:(s + 1) * P],
                                                ident[:])
                            vsb = sb1.tile([P, P], fp32, tag="vsb")
                            nc.scalar.copy(vsb[:], vtp[:])
                            nc.vector.tensor_copy(vpv[b][:, jt, 0:D],
                                                  vsb[:, 0:D])
                            nc.vector.tensor_copy(vpv[b][:, jt, PVW:PVW + D],
                                                  vsb[:, D:2 * D])
                            nc.sync.dma_start(
                                kvp.ap()[b * T + t0:b * T + t0 + P, 1, :],
                                vsb[:])
                            # k natural (current chunk) for the kv output
                            ktp = ps1.tile([P, P], fp16, tag="tp")
                            nc.tensor.transpose(
                                ktp[:],
                                kT[b][:, MXL + t0:MXL + t0 + P],
                                ident[:])
                            ksb = sb1.tile([P, P], fp32, tag="ksb")
                            nc.vector.tensor_copy(ksb[:], ktp[:])
                            nc.sync.dma_start(
                                kvp.ap()[b * T + t0:b * T + t0 + P, 0, :],
                                ksb[:])
                    if b == 0:
                        # phase-2 feed DMAs: emitted after b=0 so the first
                        # projection matmuls aren't starved by them
                        nc.gpsimd.dma_start(wo_sb[:], wo.ap())
                        kxl_r = kxlT.ap().rearrange("p (b t) -> p b t", b=B)
                        vxl_r = vxl.ap().rearrange("(b j p) c -> p b j c",
                                                   b=B, p=P)
                        for bb in range(B):
                            nc.gpsimd.dma_start(kT[bb][:, 0:MXL],
                                                kxl_r[:, bb, :])
                            nc.gpsimd.dma_start(vpv[bb][:, 0:MXL // P, 0:D],
                                                vxl_r[:, bb, :, 0:D])
                            nc.gpsimd.dma_start(
                                vpv[bb][:, 0:MXL // P, PVW:PVW + D],
                                vxl_r[:, bb, :, D:2 * D])
                            nc.vector.memset(vpv[bb][:, :, D:PVW], 1.0)
                            nc.vector.memset(vpv[bb][:, :, PVW + D:2 * PVW],
                                             1.0)
                        for h in range(HPC):
                            nc.gpsimd.dma_start(
                                er_sb[:, h],
                                erel.ap()[h].rearrange("(a p) i -> p a i",
                                                       p=P))

            # ---- Phase 2: attention, with interleaved output projection ----
            with tc.tile_pool(name="exs", bufs=3) as exsp, \
                 tc.tile_pool(name="pex", bufs=3) as pexp, \
                 tc.tile_pool(name="sm", bufs=2) as smp, \
                 tc.tile_pool(name="osb", bufs=3) as osbp, \
                 tc.tile_pool(name="psS", bufs=1, space="PSUM") as psS, \
                 tc.tile_pool(name="psPV", bufs=1, space="PSUM") as psPV, \
                 tc.tile_pool(name="psO", bufs=2, space="PSUM") as psO:
                for ir in range(T // IR):
                    i0 = ir * IR
                    nj = (MXL + i0 + IR) // P  # visible j-tiles (12 or 16)
                    for b in range(B):
                        pvps = [psPV.tile([P, IR], fp32, tag=f"pv{h}",
                                          name=f"pv{h}") for h in range(HPC)]
                        for jp in range(nj // 2):
                            sps = [psS.tile([P, 2, IR], fp32, tag=f"s{h}",
                                            name=f"s{h}") for h in range(HPC)]
                            for u in range(2):
                                jt = jp * 2 + u
                                for h in range(HPC):
                                    h0 = h * D
                                    nc.tensor.matmul(
                                        sps[h][:, u, :],
                                        kT[b][h0:h0 + D, jt * P:(jt + 1) * P],
                                        qT[b][h0:h0 + D, i0:i0 + IR],
                                        start=True, stop=True)
                            for h in range(HPC):
                                exs = exsp.tile([P, 2, IR], fp16, tag="exs")
                                nc.scalar.activation(exs[:], sps[h][:],
                                                     AF.Exp, scale=SCALE)
                                pex = pexp.tile([P, 2, IR], fp16, tag="pex")
                                nc.vector.tensor_mul(
                                    pex[:], exs[:],
                                    er_sb[:, h, jp * 2:jp * 2 + 2,
                                          i0:i0 + IR])
                                for u in range(2):
                                    jt = jp * 2 + u
                                    nc.tensor.matmul(
                                        pvps[h][:],
                                        vpv[b][:, jt,
                                               h * PVW:(h + 1) * PVW],
                                        pex[:, u, :],
                                        start=(jt == 0), stop=(jt == nj - 1))
                        for h in range(HPC):
                            h0 = h * D
                            rs = smp.tile([D, IR], fp32, tag="rs")
                            nc.vector.tensor_copy(rs[:], pvps[h][D:2 * D, :])
                            rb = smp.tile([D, IR], fp32, tag="rb")
                            nc.vector.reciprocal_approx_fast(rb[:], rs[:])
                            nc.vector.tensor_mul(
                                qkvn[b][h0:h0 + D, i0:i0 + IR],
                                pvps[h][0:D, :], rb[:])
                        # partial output projection for this (b, i-range)
                        for tt in range(IR // P):
                            t0 = i0 + tt * P
                            for eh in range(E // 512):
                                ops = psO.tile([P, 512], fp32, tag="o")
                                nc.tensor.matmul(
                                    ops[:], qkvn[b][:, t0:t0 + P],
                                    wo_sb[:, eh * 512:(eh + 1) * 512],
                                    start=True, stop=True)
                                osb = osbp.tile([P, 512], fp32, tag="osb")
                                nc.vector.tensor_copy(osb[:], ops[:])
                                nc.sync.dma_start(
                                    outp.ap()[b * T + t0:b * T + t0 + P,
                                              eh * 512:(eh + 1) * 512],
                                    osb[:])

    nc.compile()
    return nc


def _get_program():
    if "nc" not in _CACHE:
        _CACHE["nc"] = _build_program()
    return _CACHE["nc"]


def _prep_inputs(x, xl, rel, Wq, bq, Wk, bk, Wv, bv, Wo):
    """Host-side sharding/layout prep. Returns per-core input maps."""
    f16 = np.float16
    xT = np.ascontiguousarray(x.reshape(BT, E).T).astype(f16)   # [E, BT]

    # mask (j >= i + MXL + 1) and rel bias folded into exp(rel * SCALE)
    jj = np.arange(J, dtype=np.int64)[:, None]
    ii = np.arange(T, dtype=np.int64)[None, :]
    maskT = jj >= (ii + MXL + 1)                                # [J, T]

    in_maps = []
    for c in range(NCORES):
        cs = slice(c * HDC, (c + 1) * HDC)
        relc = np.exp(rel[c * HPC:(c + 1) * HPC].transpose(0, 2, 1) * SCALE)
        relc[:, maskT] = 0.0
        in_maps.append({
            "xTd": xT,
            "wq": np.ascontiguousarray(Wq[:, cs] * SCALE).astype(f16),
            "wk": np.ascontiguousarray(Wk[:, cs]).astype(f16),
            "wv": np.ascontiguousarray(Wv[:, cs]).astype(f16),
            "wo": np.ascontiguousarray(Wo[cs, :]).astype(f16),
            "bqd": np.ascontiguousarray(
                (bq[cs] * SCALE).reshape(HDC, 1).astype(np.float32)),
            "bkd": np.ascontiguousarray(bk[cs].reshape(HDC, 1)),
            "bvd": np.ascontiguousarray(bv[cs].reshape(HDC, 1)),
            "kxlT": np.ascontiguousarray(
                xl[:, :, 0, cs].reshape(BT, HDC).T).astype(f16),
            "vxl": np.ascontiguousarray(
                xl[:, :, 1, cs].reshape(BT, HDC)).astype(f16),
            "erel": np.ascontiguousarray(relc).astype(f16),
        })
    return in_maps


def _run(inputs, trace=False, tmpdir=None, trace_cores=None):
    from concourse.bass_utils import run_bass_kernel_spmd

    f = lambda k: np.asarray(inputs[k], np.float32)
    in_maps = _prep_inputs(f("x"), f("xl_memory"), f("relative_positions"),
                           f("Wq"), f("bq"), f("Wk"), f("bk"),
                           f("Wv"), f("bv"), f("Wo"))
    bo = f("bo")

    nc = _get_program()
    kw = {}
    if trace:
        kw.update(trace=True, tmpdir=tmpdir, trace_cores=trace_cores)
    res = run_bass_kernel_spmd(nc, in_maps, list(range(NCORES)), **kw)

    out = np.zeros((BT, E), np.float32)
    kv = np.empty((B, T, 2, H * D), np.float32)
    for c in range(NCORES):
        cs = slice(c * HDC, (c + 1) * HDC)
        out += res.results[c]["outp"]
        kv[:, :, :, cs] = res.results[c]["kvp"].reshape(B, T, 2, HDC)
    out = out.reshape(B, T, E) + bo
    return (out, kv), res


def kernel(**inputs):
    outs, _ = _run(inputs, trace=False)
    return outs
